# revision 20
# baseline (speedup 1.0000x reference)
"""Locally-connected network (28x28 -> lc3x3 -> lc3x3 -> fc10) on 8 TRN2 cores.

The whole reference network is linear (two locally-connected layers + FC, no
activations), so on the host we fold it into a single affine map
    out[b, :] = x[b, :784] @ M + c          (M: [784, 10], c: [10])
computed in float64. The device kernel is a pure data-parallel, memory-bound
matmul in bf16 (tolerance is 2e-2; bf16 in/out contributes ~3e-3).

Per core (1024 samples), one DRAM tensor packs everything in bf16:
    xt[112, 128 + 7*1024]: cols 0..127 = weights (70 cols of M-blocks,
    f32 bias bits at cols 96..97), then 7 k-tiles x 1024 batch columns.
Loads are one DMA per k-tile (2KB contiguous per partition) alternating
between the two HWDGE rings (sync/scalar) so transfers pipeline at the
~260 GB/s aggregate DMA ceiling; the first load also carries the weight
block (keeping its descriptors large — a separate small-descriptor weight
load at a queue head skews the 16 DMA engines and straggles every later
semaphore). The last k-tile and the output store are split by batch chunk
so chunk 0's bias + store overlap chunk 1's stream and compute. The two
output stores are emitted last, so they take the two reused DMAHW
semaphore lanes (10 DMAs > 8 lanes) after their lanes' prior transfers
have long completed.

PE does 7 accumulating bf16 matmuls per 512-sample chunk into two PSUM
banks, interleaved across chunks to consume k-tiles as they arrive; a few
throwaway matmuls on an unwritten scratch tile keep PE busy from block
start so it is out of the low DVFS p-state when real data lands. Bias-add
+ PSUM->SBUF copy runs on the scalar engine for chunk 0 and DVE for chunk
1 (parallel), writing bf16; the host upcasts to f32.
"""

import numpy as np
import ml_dtypes

import concourse.bass as bass
import concourse.tile as tile
from concourse import bacc, mybir
from concourse.bass_utils import run_bass_kernel_spmd

N_CORES = 8
B = 8192
B_SHARD = B // N_CORES          # 1024
PIX = 784                       # 28*28
KP = 112                        # K-tile partition count; 784 = 7 * 112
NKT = PIX // KP                 # 7
NCHUNK = 2                      # batch chunks of 512 (PSUM bank = 512 f32)
CH = B_SHARD // NCHUNK          # 512
NOUT = 10
MW_COLS = 128                   # cols 0..69 = M blocks, 96..97 = f32 bias bits
BIAS_COL = 96                   # even (4-byte aligned) for the f32 bitcast
XT_COLS = MW_COLS + NKT * B_SHARD
N_DUMMY = 0                     # PE warm-up matmuls hurt: they contend with
                                # DMA SBUF writes and slow the stream ~1us

BF16 = ml_dtypes.bfloat16


def _lc_dense(w, H, W_, oh, ow):
    """Dense [H*W_, oh*ow] matrix of one 3x3 locally-connected layer."""
    w = np.asarray(w, np.float64).reshape(oh, ow, 9)
    M = np.zeros((H * W_, oh * ow), np.float64)
    ox, oy = np.meshgrid(np.arange(oh), np.arange(ow), indexing="ij")
    col = (ox * ow + oy).ravel()
    for i in range(3):
        for j in range(3):
            row = ((ox + i) * W_ + (oy + j)).ravel()
            M[row, col] += w[:, :, i * 3 + j].ravel()
    return M


def _fold(w1, b1, w2, b2, fc_w, fc_b):
    W1 = _lc_dense(w1, 28, 28, 26, 26)          # [784, 676]
    W2 = _lc_dense(w2, 26, 26, 24, 24)          # [676, 576]
    fcw = np.asarray(fc_w, np.float64)          # [10, 576]
    M = W1 @ W2 @ fcw.T                         # [784, 10]
    c = (
        np.asarray(b1, np.float64).reshape(-1) @ W2
        + np.asarray(b2, np.float64).reshape(-1)
    ) @ fcw.T + np.asarray(fc_b, np.float64)    # [10]
    return M.astype(np.float32), c.astype(np.float32)


def _build_bass(n_dummy=N_DUMMY, split_tail=True, bf16_out=True):
    nc = bacc.Bacc("TRN2", target_bir_lowering=False, debug=False)
    xt = nc.declare_dram_parameter("xt", [KP, XT_COLS], mybir.dt.bfloat16, isOutput=False)
    out_dt = mybir.dt.bfloat16 if bf16_out else mybir.dt.float32
    out = nc.declare_dram_parameter("out", [NOUT, B_SHARD], out_dt, isOutput=True)

    with tile.TileContext(nc) as tc:
        with (
            tc.tile_pool(name="xp", bufs=1) as xp,
            tc.tile_pool(name="pp", bufs=1, space="PSUM") as pp,
            tc.tile_pool(name="op", bufs=1) as op,
        ):
            rings = [nc.sync, nc.scalar]
            # k-tile 0 rides with the weight block in one load (sync ring).
            t0 = xp.tile([KP, MW_COLS + B_SHARD], mybir.dt.bfloat16)
            nc.sync.dma_start(t0[:], xt[:, 0 : MW_COLS + B_SHARD])
            xts = [(t0, MW_COLS)]
            n_full = NKT - 1 if split_tail else NKT
            for kt in range(1, n_full):
                t = xp.tile([KP, B_SHARD], mybir.dt.bfloat16, name=f"t{kt}")
                c0 = MW_COLS + kt * B_SHARD
                rings[kt % 2].dma_start(t[:], xt[:, c0 : c0 + B_SHARD])
                xts.append((t, 0))
            tl = []
            if split_tail:
                for c in range(NCHUNK):
                    t = xp.tile([KP, CH], mybir.dt.bfloat16, name=f"t6c{c}")
                    c0 = MW_COLS + (NKT - 1) * B_SHARD + c * CH
                    rings[c % 2].dma_start(t[:], xt[:, c0 : c0 + CH])
                    tl.append(t)

            bias_ap = t0[0:NOUT, BIAS_COL : BIAS_COL + 2].bitcast(mybir.dt.float32)

            # PE p-state warm-up: matmuls on a never-written scratch tile run
            # unguarded from block start, so the DVFS ramp (full speed after
            # ~3us of continuous execution) is underway before data arrives.
            if n_dummy:
                warm = pp.tile([NOUT, CH], mybir.dt.float32)
                junk = nc.alloc_sbuf_tensor("junk", [KP, CH], mybir.dt.bfloat16).ap()
                for _ in range(n_dummy):
                    nc.tensor.matmul(
                        warm[:], junk[:, 0:NOUT], junk[:, :], start=True, stop=True
                    )

            # The bias-add engines read both PSUM (PE sem) and t0 (DMA lane);
            # touch t0 here so the real op waits on at most one sem (TRN2
            # codegen allows a single sync wait per instruction).
            scr_v = op.tile([1, 1], mybir.dt.float32)
            nc.vector.tensor_scalar_add(scr_v[:], bias_ap[0:1, :], 0.0)
            scr_s = op.tile([1, 1], mybir.dt.float32)
            nc.scalar.copy(scr_s[:], bias_ap[0:1, :])

            ps = [
                pp.tile([NOUT, CH], mybir.dt.float32, name=f"ps{c}")
                for c in range(NCHUNK)
            ]
            o = op.tile([NOUT, B_SHARD], out_dt)
            for kt in range(NKT):
                w_ap = t0[:, kt * NOUT : (kt + 1) * NOUT]
                for c in range(NCHUNK):
                    if split_tail and kt == NKT - 1:
                        rhs = tl[c][:, :]
                    else:
                        t, base = xts[kt]
                        rhs = t[:, base + c * CH : base + (c + 1) * CH]
                    nc.tensor.matmul(
                        ps[c][:], w_ap, rhs, start=(kt == 0), stop=(kt == NKT - 1)
                    )
                    if kt == NKT - 1:
                        # Bias-add + PSUM->SBUF as quarter-chunks ping-ponged
                        # across scalar and DVE: each chunk's halves run in
                        # parallel the moment its stop-matmul retires, and
                        # chunk 0 (whose data lands first) goes first on both
                        # engines. ~0.47us per 256-col op vs ~0.7 for 512.
                        h = CH // 2
                        oc = o[:, c * CH : (c + 1) * CH]
                        e0, e1 = (
                            (0, 1) if c == 0 else (1, 0)
                        )  # alternate engine taking the low half
                        halves = [(0, h, e0), (h, CH, e1)]
                        for lo, hi, eng in halves:
                            if eng == 0:
                                nc.scalar.activation(
                                    oc[:, lo:hi],
                                    ps[c][:, lo:hi],
                                    mybir.ActivationFunctionType.Identity,
                                    bias=bias_ap,
                                )
                            else:
                                nc.vector.tensor_scalar_add(
                                    oc[:, lo:hi], ps[c][:, lo:hi], bias_ap
                                )
            # Single store on the sync ring: its DMA issue is ~450ns faster
            # than scalar's, and both chunks' biases finish within ~0.3us of
            # each other anyway (the stream delivers the split last k-tile
            # halves nearly together).
            nc.sync.dma_start(out[:], o[:])
    nc.finalize()
    return nc


def _pack_inputs(inputs):
    x = np.asarray(inputs["x"], np.float32)
    M, c = _fold(
        inputs["w1"], inputs["b1"], inputs["w2"], inputs["b2"],
        inputs["fc_w"], inputs["fc_b"],
    )
    # Weight block: cols kt*10..kt*10+9 = M[kt*112:(kt+1)*112, :] in bf16;
    # cols 96..97 hold the f32 bias bit pattern (read via bitcast on device).
    mp = np.zeros((KP, MW_COLS), BF16)
    mp[:, : NKT * NOUT] = (
        M.astype(BF16).reshape(NKT, KP, NOUT).transpose(1, 0, 2).reshape(KP, NKT * NOUT)
    )
    mp.view(np.uint16)[0:NOUT, BIAS_COL : BIAS_COL + 2] = (
        c.astype("<f4").view("<u2").reshape(NOUT, 2)
    )

    # Per-core x shard packed to bf16 [KP, NKT*B_SHARD]:
    # col kt*1024 + b = x[b, kt*112 + p].
    xr = x.reshape(B, PIX).astype(BF16)
    in_maps = []
    for i in range(N_CORES):
        xs = (
            xr[i * B_SHARD : (i + 1) * B_SHARD]
            .reshape(B_SHARD, NKT, KP)
            .transpose(2, 1, 0)
            .reshape(KP, NKT * B_SHARD)
        )
        in_maps.append({"xt": np.ascontiguousarray(np.hstack([mp, xs]))})
    return in_maps


def _run(inputs, trace=False, trace_cores=None, **build_opts):
    in_maps = _pack_inputs(inputs)
    nc = _build_bass(**build_opts)
    res = run_bass_kernel_spmd(
        nc,
        in_maps,
        list(range(N_CORES)),
        trace=trace,
        trace_cores=trace_cores,
    )
    out = np.concatenate(
        [np.asarray(res.results[i]["out"]).astype(np.float32).T for i in range(N_CORES)],
        axis=0,
    )
    return out, res


def kernel(**inputs) -> np.ndarray:
    out, _ = _run(inputs, trace=False)
    return out


# revision 21
# speedup vs baseline: 1.1234x; 1.1234x over previous
"""Locally-connected network (28x28 -> lc3x3 -> lc3x3 -> fc10) on 8 TRN2 cores.

The whole reference network is linear (two locally-connected layers + FC, no
activations), so on the host we fold it into a single affine map
    out[b, :] = x[b, :784] @ M + c          (M: [784, 10], c: [10])
computed in float64. The device kernel is a pure data-parallel, memory-bound
matmul in bf16 (tolerance is 2e-2; bf16 in/out contributes ~3e-3).

Per core (1024 samples), one DRAM tensor packs everything in bf16:
    xt[112, 128 + 7*1024]: cols 0..127 = weights (70 cols of M-blocks,
    f32 bias bits at cols 96..97), then 7 k-tiles x 1024 batch columns.
Loads are one DMA per k-tile (2KB contiguous per partition) alternating
between the two HWDGE rings (sync/scalar) so transfers pipeline at the
~260 GB/s aggregate DMA ceiling; the first load also carries the weight
block (keeping its descriptors large — a separate small-descriptor weight
load at a queue head skews the 16 DMA engines and straggles every later
semaphore). The last k-tile is split by batch chunk so each chunk's
stop-matmul retires as soon as its half of the stream lands.

PE does 7 accumulating bf16 matmuls per 512-sample chunk into two PSUM
banks, interleaved across chunks to consume k-tiles as they arrive.
Bias-add + PSUM->SBUF copy (the only engines that can read PSUM are
scalar and DVE) runs as quarter-chunks ping-ponged across both engines,
writing bf16; one store on the sync ring (fastest DMA issue) writes
[10, 1024], and the host upcasts to f32. PE p-state warm-up matmuls were
tried and removed: their SBUF reads contend with DMA writes and slow the
stream more than the faster matmuls gain.
"""

import numpy as np
import ml_dtypes

import concourse.bass as bass
import concourse.tile as tile
from concourse import bacc, mybir
from concourse.bass_utils import run_bass_kernel_spmd

N_CORES = 8
B = 8192
B_SHARD = B // N_CORES          # 1024
PIX = 784                       # 28*28
KP = 112                        # K-tile partition count; 784 = 7 * 112
NKT = PIX // KP                 # 7
NCHUNK = 2                      # batch chunks of 512 (PSUM bank = 512 f32)
CH = B_SHARD // NCHUNK          # 512
NOUT = 10
MW_COLS = 128                   # cols 0..69 = M blocks, 96..97 = f32 bias bits
BIAS_COL = 96                   # even (4-byte aligned) for the f32 bitcast
XT_COLS = MW_COLS + NKT * B_SHARD
N_DUMMY = 0                     # PE warm-up matmuls hurt: they contend with
                                # DMA SBUF writes and slow the stream ~1us

BF16 = ml_dtypes.bfloat16


def _lc_dense(w, H, W_, oh, ow):
    """Dense [H*W_, oh*ow] matrix of one 3x3 locally-connected layer."""
    w = np.asarray(w, np.float64).reshape(oh, ow, 9)
    M = np.zeros((H * W_, oh * ow), np.float64)
    ox, oy = np.meshgrid(np.arange(oh), np.arange(ow), indexing="ij")
    col = (ox * ow + oy).ravel()
    for i in range(3):
        for j in range(3):
            row = ((ox + i) * W_ + (oy + j)).ravel()
            M[row, col] += w[:, :, i * 3 + j].ravel()
    return M


def _fold(w1, b1, w2, b2, fc_w, fc_b):
    W1 = _lc_dense(w1, 28, 28, 26, 26)          # [784, 676]
    W2 = _lc_dense(w2, 26, 26, 24, 24)          # [676, 576]
    fcw = np.asarray(fc_w, np.float64)          # [10, 576]
    M = W1 @ W2 @ fcw.T                         # [784, 10]
    c = (
        np.asarray(b1, np.float64).reshape(-1) @ W2
        + np.asarray(b2, np.float64).reshape(-1)
    ) @ fcw.T + np.asarray(fc_b, np.float64)    # [10]
    return M.astype(np.float32), c.astype(np.float32)


def _build_bass(n_dummy=N_DUMMY, split_tail=True, bf16_out=True):
    nc = bacc.Bacc("TRN2", target_bir_lowering=False, debug=False)
    xt = nc.declare_dram_parameter("xt", [KP, XT_COLS], mybir.dt.bfloat16, isOutput=False)
    out_dt = mybir.dt.bfloat16 if bf16_out else mybir.dt.float32
    out = nc.declare_dram_parameter("out", [NOUT, B_SHARD], out_dt, isOutput=True)

    with tile.TileContext(nc) as tc:
        with (
            tc.tile_pool(name="xp", bufs=1) as xp,
            tc.tile_pool(name="pp", bufs=1, space="PSUM") as pp,
            tc.tile_pool(name="op", bufs=1) as op,
        ):
            rings = [nc.sync, nc.scalar]
            # k-tile 0 rides with the weight block in one load (sync ring).
            t0 = xp.tile([KP, MW_COLS + B_SHARD], mybir.dt.bfloat16)
            nc.sync.dma_start(t0[:], xt[:, 0 : MW_COLS + B_SHARD])
            xts = [(t0, MW_COLS)]
            n_full = NKT - 1 if split_tail else NKT
            for kt in range(1, n_full):
                t = xp.tile([KP, B_SHARD], mybir.dt.bfloat16, name=f"t{kt}")
                c0 = MW_COLS + kt * B_SHARD
                rings[kt % 2].dma_start(t[:], xt[:, c0 : c0 + B_SHARD])
                xts.append((t, 0))
            tl = []
            if split_tail:
                for c in range(NCHUNK):
                    t = xp.tile([KP, CH], mybir.dt.bfloat16, name=f"t6c{c}")
                    c0 = MW_COLS + (NKT - 1) * B_SHARD + c * CH
                    rings[c % 2].dma_start(t[:], xt[:, c0 : c0 + CH])
                    tl.append(t)

            bias_ap = t0[0:NOUT, BIAS_COL : BIAS_COL + 2].bitcast(mybir.dt.float32)

            # PE p-state warm-up: matmuls on a never-written scratch tile run
            # unguarded from block start, so the DVFS ramp (full speed after
            # ~3us of continuous execution) is underway before data arrives.
            if n_dummy:
                warm = pp.tile([NOUT, CH], mybir.dt.float32)
                junk = nc.alloc_sbuf_tensor("junk", [KP, CH], mybir.dt.bfloat16).ap()
                for _ in range(n_dummy):
                    nc.tensor.matmul(
                        warm[:], junk[:, 0:NOUT], junk[:, :], start=True, stop=True
                    )

            # The bias-add engines read both PSUM (PE sem) and t0 (DMA lane);
            # touch t0 here so the real op waits on at most one sem (TRN2
            # codegen allows a single sync wait per instruction).
            scr_v = op.tile([1, 1], mybir.dt.float32)
            nc.vector.tensor_scalar_add(scr_v[:], bias_ap[0:1, :], 0.0)
            scr_s = op.tile([1, 1], mybir.dt.float32)
            nc.scalar.copy(scr_s[:], bias_ap[0:1, :])

            ps = [
                pp.tile([NOUT, CH], mybir.dt.float32, name=f"ps{c}")
                for c in range(NCHUNK)
            ]
            o = op.tile([NOUT, B_SHARD], out_dt)
            for kt in range(NKT):
                w_ap = t0[:, kt * NOUT : (kt + 1) * NOUT]
                for c in range(NCHUNK):
                    if split_tail and kt == NKT - 1:
                        rhs = tl[c][:, :]
                    else:
                        t, base = xts[kt]
                        rhs = t[:, base + c * CH : base + (c + 1) * CH]
                    nc.tensor.matmul(
                        ps[c][:], w_ap, rhs, start=(kt == 0), stop=(kt == NKT - 1)
                    )
                    if kt == NKT - 1:
                        # Bias-add + PSUM->SBUF as quarter-chunks ping-ponged
                        # across scalar and DVE: each chunk's halves run in
                        # parallel the moment its stop-matmul retires, and
                        # chunk 0 (whose data lands first) goes first on both
                        # engines. ~0.47us per 256-col op vs ~0.7 for 512.
                        h = CH // 2
                        oc = o[:, c * CH : (c + 1) * CH]
                        e0, e1 = (
                            (0, 1) if c == 0 else (1, 0)
                        )  # alternate engine taking the low half
                        halves = [(0, h, e0), (h, CH, e1)]
                        for lo, hi, eng in halves:
                            if eng == 0:
                                nc.scalar.activation(
                                    oc[:, lo:hi],
                                    ps[c][:, lo:hi],
                                    mybir.ActivationFunctionType.Identity,
                                    bias=bias_ap,
                                )
                            else:
                                nc.vector.tensor_scalar_add(
                                    oc[:, lo:hi], ps[c][:, lo:hi], bias_ap
                                )
            # Single store on the sync ring: its DMA issue is ~450ns faster
            # than scalar's, and both chunks' biases finish within ~0.3us of
            # each other anyway (the stream delivers the split last k-tile
            # halves nearly together).
            nc.sync.dma_start(out[:], o[:])
    nc.finalize()
    return nc


def _pack_inputs(inputs):
    x = np.asarray(inputs["x"], np.float32)
    M, c = _fold(
        inputs["w1"], inputs["b1"], inputs["w2"], inputs["b2"],
        inputs["fc_w"], inputs["fc_b"],
    )
    # Weight block: cols kt*10..kt*10+9 = M[kt*112:(kt+1)*112, :] in bf16;
    # cols 96..97 hold the f32 bias bit pattern (read via bitcast on device).
    mp = np.zeros((KP, MW_COLS), BF16)
    mp[:, : NKT * NOUT] = (
        M.astype(BF16).reshape(NKT, KP, NOUT).transpose(1, 0, 2).reshape(KP, NKT * NOUT)
    )
    mp.view(np.uint16)[0:NOUT, BIAS_COL : BIAS_COL + 2] = (
        c.astype("<f4").view("<u2").reshape(NOUT, 2)
    )

    # Per-core x shard packed to bf16 [KP, NKT*B_SHARD]:
    # col kt*1024 + b = x[b, kt*112 + p].
    xr = x.reshape(B, PIX).astype(BF16)
    in_maps = []
    for i in range(N_CORES):
        xs = (
            xr[i * B_SHARD : (i + 1) * B_SHARD]
            .reshape(B_SHARD, NKT, KP)
            .transpose(2, 1, 0)
            .reshape(KP, NKT * B_SHARD)
        )
        in_maps.append({"xt": np.ascontiguousarray(np.hstack([mp, xs]))})
    return in_maps


def _run(inputs, trace=False, trace_cores=None, **build_opts):
    in_maps = _pack_inputs(inputs)
    nc = _build_bass(**build_opts)
    res = run_bass_kernel_spmd(
        nc,
        in_maps,
        list(range(N_CORES)),
        trace=trace,
        trace_cores=trace_cores,
    )
    out = np.concatenate(
        [np.asarray(res.results[i]["out"]).astype(np.float32).T for i in range(N_CORES)],
        axis=0,
    )
    return out, res


def kernel(**inputs) -> np.ndarray:
    out, _ = _run(inputs, trace=False)
    return out
